# revision 5
# baseline (speedup 1.0000x reference)
"""Multi-head attention kernel for Trainium2, batch-parallel across 8 NeuronCores.

Reference (per batch element b, one core each):
  qk = x @ W_qk.T ; q,k = split(qk) ; v = x @ W_v.T
  q,k,v -> [h, n, d] ; q += pos_h ; k += pos_h
  S = q @ k.T * DIM**-0.5 ; mask = outer(m, m) ; masked -> -inf
  P = softmax(S) ; O = P @ v ; out = merge_heads(O) @ W_out.T + b_out

Device strategy (per core):
  - everything in fp32r (TF32-like, full PE rate at free-dim >= 256)
  - x, pos, W_qk, W_v, W_out transposed on-chip via PE-transpose
  - scores computed TRANSPOSED: ST[j, i] = k_h @ q_h.T so the scale and the
    column mask fold into one ACT exp (bias = per-partition mask bias) and
    no vector reductions are needed: softmax row sums come from an appended
    ones-column in the PV matmul (V_aug = [V_h | 1], M=65).
  - normalization (1/s), row mask, and masked-row blend with mean(V) are
    applied per head on the PV output with 2 DVE ops.
  - out-projection accumulates over head dims in PSUM and adds the bias via
    a broadcast tensor during the PSUM->SBUF copy.
"""
import sys

sys.path.insert(0, "/opt/trn_rl_repo")

import numpy as np
from contextlib import ExitStack

B, N, DIM, H = 8, 1024, 1024, 16
D = DIM // H          # 64
E = D + 1             # V_aug block (64 cols of V + ones column)
P = 128
NT = N // P           # 8 n-tiles
KT = DIM // P         # 8 k-tiles
NPAIR = H // 2        # 8 head pairs
SCALE = DIM ** (-0.5)
MB = 30.0             # mask bias magnitude: bias_j = 30*m - 30 in {0, -30}

_NC = None


def _build():
    import concourse.bacc as bacc
    import concourse.bass as bass
    import concourse.mybir as mybir
    import concourse.tile as tile
    from concourse.masks import make_identity

    f32 = mybir.dt.float32
    f32r = mybir.dt.float32r
    AF = mybir.ActivationFunctionType
    OP = mybir.AluOpType
    ts = bass.ts

    nc = bacc.Bacc()
    x_d = nc.declare_dram_parameter("x", [N, DIM], f32, isOutput=False)
    pos_d = nc.declare_dram_parameter("pos", [N, DIM], f32, isOutput=False)
    maskf_d = nc.declare_dram_parameter("maskf", [N], f32, isOutput=False)
    wqk_d = nc.declare_dram_parameter("W_qk", [2 * DIM, DIM], f32, isOutput=False)
    wv_d = nc.declare_dram_parameter("W_v", [DIM, DIM], f32, isOutput=False)
    wout_d = nc.declare_dram_parameter("W_out", [DIM, DIM], f32, isOutput=False)
    b_d = nc.declare_dram_parameter("b_out", [DIM], f32, isOutput=False)
    out_d = nc.declare_dram_parameter("out", [N, DIM], f32, isOutput=True)

    with ExitStack() as ctx:
        tc = ctx.enter_context(tile.TileContext(nc))
        sing = ctx.enter_context(tc.tile_pool(name="sing", bufs=1))
        rowload = ctx.enter_context(tc.tile_pool(name="rowload", bufs=2))
        pair2 = ctx.enter_context(tc.tile_pool(name="pair2", bufs=2))
        work1 = ctx.enter_context(tc.tile_pool(name="work1", bufs=1))
        work2 = ctx.enter_context(tc.tile_pool(name="work2", bufs=2))
        expp = ctx.enter_context(tc.tile_pool(name="expp", bufs=2))
        otrp = ctx.enter_context(tc.tile_pool(name="otrp", bufs=10))
        dramp = ctx.enter_context(tc.tile_pool(name="dramp", bufs=1, space="DRAM"))
        ps_mm = ctx.enter_context(tc.tile_pool(name="ps_mm", bufs=2, space="PSUM"))
        ps_tp = ctx.enter_context(tc.tile_pool(name="ps_tp", bufs=2, space="PSUM"))
        ps_st = ctx.enter_context(tc.tile_pool(name="ps_st", bufs=1, space="PSUM"))
        ps_oa = ctx.enter_context(tc.tile_pool(name="ps_oa", bufs=2, space="PSUM"))

        # ---------- constants / small prep ----------
        ident = sing.tile([P, P], f32, tag="ident")
        make_identity(nc, ident)

        # maskf in [p, c] layout (j = c*128 + p): exp bias column per j-tile
        mstage = sing.tile([P, NT], f32, tag="mstage")
        nc.sync.dma_start(out=mstage, in_=maskf_d.rearrange("(c p) -> p c", c=NT))
        bias_j = sing.tile([P, NT], f32, tag="bias_j")
        nc.scalar.activation(bias_j, mstage, AF.Copy, bias=-MB, scale=MB)
        # mask rows
        m_row = sing.tile([1, N], f32, tag="m_row")
        nc.sync.dma_start(out=m_row, in_=maskf_d[:])
        omm_row = sing.tile([1, N], f32, tag="omm_row")
        nc.vector.tensor_scalar(omm_row, m_row, -1.0, 1.0, OP.mult, OP.add)
        ommb = sing.tile([P, N], f32, tag="ommb")
        nc.gpsimd.partition_broadcast(ommb, omm_row)
        # mask in the s-collect layout: m_coll[p, i*4+c] = maskf[i*512 + p*4 + c]
        m_coll = sing.tile([P, 8], f32, tag="m_coll")
        nc.sync.dma_start(
            out=m_coll.rearrange("p (i c) -> p i c", i=2),
            in_=maskf_d.rearrange("(i p c) -> p i c", i=2, c=4),
        )
        # bias row -> broadcast over partitions
        b_row = sing.tile([1, DIM], f32, tag="b_row")
        nc.sync.dma_start(out=b_row, in_=b_d[:])
        b_bcast = sing.tile([P, DIM], f32, tag="b_bcast")
        nc.gpsimd.partition_broadcast(b_bcast, b_row)

        # ---------- x^T ----------
        xT = [sing.tile([P, N], f32r, tag=f"xT{kc}", name=f"xT{kc}") for kc in range(KT)]
        for nt in range(NT):
            xr = rowload.tile([P, DIM], f32, tag="rowload")
            nc.sync.dma_start(out=xr, in_=x_d[ts(nt, P), :])
            for kc in range(KT):
                tp = ps_tp.tile([P, P], f32, tag="tp")
                nc.tensor.transpose(tp, xr[:, ts(kc, P)], ident)
                nc.vector.tensor_copy(xT[kc][:, ts(nt, P)], tp)

        # ---------- V = x @ W_v.T  (stored as [V_h | 1] x 16 heads) ----------
        V_sb = [sing.tile([P, H * E], f32r, tag=f"V{nt}", name=f"V{nt}") for nt in range(NT)]
        const1 = sing.tile([P, H], f32, tag="const1")
        nc.vector.memset(const1, 1.0)
        for nt in range(NT):
            ones_ap = V_sb[nt].rearrange("p (h e) -> p h e", e=E)[:, :, D:E]
            nc.vector.tensor_copy(ones_ap.squeeze(), const1)
        whalf = [None] * KT
        for dvh in range(2):
            for kc in range(KT):
                whalf[kc] = work1.tile([P, 512], f32r, tag=f"whalf{kc}", name=f"whalf{kc}")
            for rt in range(4):
                wr = rowload.tile([P, DIM], f32, tag="rowload")
                nc.sync.dma_start(
                    out=wr,
                    in_=wv_d[dvh * 512 + rt * P: dvh * 512 + (rt + 1) * P, :])
                for kc in range(KT):
                    tp = ps_tp.tile([P, P], f32, tag="tp")
                    nc.tensor.transpose(tp, wr[:, ts(kc, P)], ident)
                    nc.vector.tensor_copy(whalf[kc][:, ts(rt, P)], tp)
            for nt in range(NT):
                acc = ps_mm.tile([P, 512], f32, tag="mm")
                for kc in range(KT):
                    nc.tensor.matmul(acc, xT[kc][:, ts(nt, P)], whalf[kc],
                                     start=(kc == 0), stop=(kc == KT - 1))
                dst = V_sb[nt][:, dvh * 8 * E:].rearrange(
                    "p (h e) -> p h e", e=E)[:, 0:8, 0:D]
                nc.vector.tensor_copy(dst, acc.rearrange("p (h e) -> p h e", e=D))

        # ---------- mean over sequence of V_aug ----------
        ones_col = sing.tile([P, 1], f32r, tag="ones_col")
        constN = sing.tile([P, 1], f32, tag="constN")
        nc.vector.memset(constN, 1.0 / N)
        nc.vector.tensor_copy(ones_col, constN)
        mean_sb = sing.tile([1, H * E], f32, tag="mean_sb")
        for c0, cs in ((0, 512), (512, 512), (1024, H * E - 1024)):
            mp = ps_mm.tile([P, 512], f32, tag="mm")
            for nt in range(NT):
                nc.tensor.matmul(mp[0:1, 0:cs], ones_col, V_sb[nt][:, c0:c0 + cs],
                                 start=(nt == 0), stop=(nt == NT - 1))
            nc.vector.tensor_copy(mean_sb[:, c0:c0 + cs], mp[0:1, 0:cs])
        # per-head mean as a per-partition scalar column [64, H]
        mean_cols = sing.tile([D, H], f32, tag="mean_cols")
        for h in range(H):
            nc.sync.dma_start(out=mean_cols[:, h:h + 1],
                              in_=mean_sb[0:1, h * E:h * E + D])

        # ---------- per head-pair: projections + attention ----------
        ot_dram = dramp.tile([DIM, N], f32r, tag="ot")

        for t in range(NPAIR):
            # --- pos^T for this dim-slice ---
            posT = pair2.tile([P, N], f32, tag="posT")
            for nt in range(NT):
                pr = rowload.tile([P, P], f32, tag="posload")
                nc.sync.dma_start(out=pr, in_=pos_d[ts(nt, P), ts(t, P)])
                tp = ps_tp.tile([P, P], f32, tag="tp")
                nc.tensor.transpose(tp, pr, ident)
                nc.vector.tensor_copy(posT[:, ts(nt, P)], tp)
            # --- q^T / k^T for this pair (heads 2t, 2t+1) ---
            qT = pair2.tile([P, N], f32r, tag="qT")
            kT = pair2.tile([P, N], f32r, tag="kT")
            for which, wt in ((0, qT), (1, kT)):
                wqr = rowload.tile([P, DIM], f32, tag="rowload")
                nc.sync.dma_start(
                    out=wqr,
                    in_=wqk_d[which * DIM + t * P: which * DIM + (t + 1) * P, :])
                wtr = work2.tile([P, DIM], f32r, tag=f"wqkT{which}")
                for kc in range(KT):
                    tp = ps_tp.tile([P, P], f32, tag="tp")
                    nc.tensor.transpose(tp, wqr[:, ts(kc, P)], ident)
                    nc.vector.tensor_copy(wtr[:, ts(kc, P)], tp)
                for half in range(2):
                    acc = ps_mm.tile([P, 512], f32, tag="mm")
                    for kc in range(KT):
                        nc.tensor.matmul(acc, wtr[:, ts(kc, P)],
                                         xT[kc][:, ts(half, 512)],
                                         start=(kc == 0), stop=(kc == KT - 1))
                    nc.vector.tensor_add(wt[:, ts(half, 512)], acc,
                                         posT[:, ts(half, 512)])

            # --- attention, one head at a time ---
            for hs in range(2):
                h = 2 * t + hs
                hoff = hs * D
                oa = [ps_oa.tile([P, 512], f32, tag="oa", name="oa") for _ in range(2)]
                for jt in range(NT):
                    st = ps_st.tile([P, N], f32, tag="st")
                    for ih in range(2):
                        nc.tensor.matmul(st[:, ts(ih, 512)],
                                         kT[hoff:hoff + D, ts(jt, P)],
                                         qT[hoff:hoff + D, ts(ih, 512)],
                                         start=True, stop=True)
                    ex = expp.tile([P, N], f32r, tag="ex")
                    nc.scalar.activation(ex, st, AF.Exp,
                                         bias=bias_j[:, jt:jt + 1], scale=SCALE)
                    va = V_sb[jt][:, h * E:(h + 1) * E]
                    for ih in range(2):
                        nc.tensor.matmul(oa[ih][0:E, :], va, ex[:, ts(ih, 512)],
                                         start=(jt == 0), stop=(jt == NT - 1))
                # s rows: psum row 64 -> sbuf -> [p, c] collect layout
                s_stage = work1.tile([P, N], f32, tag="s_stage")
                s_coll = work2.tile([P, 8], f32, tag="s_coll")
                for ih in range(2):
                    nc.vector.tensor_copy(s_stage[D:D + 1, ts(ih, 512)],
                                          oa[ih][D:D + 1, :])
                    # s_coll[p, i*4+c] = s[i*512 + p*4 + c]
                    nc.sync.dma_start(out=s_coll[:, ih * 4:(ih + 1) * 4],
                                      in_=s_stage[D:D + 1, ts(ih, 512)])
                r_coll = work2.tile([P, 8], f32, tag="r_coll")
                nc.vector.reciprocal(r_coll, s_coll)
                nc.vector.tensor_mul(r_coll, r_coll, m_coll)
                rm_row = work2.tile([1, N], f32, tag="rm_row")
                for ih in range(2):
                    nc.sync.dma_start(
                        out=rm_row[:, ts(ih, 512)].rearrange(
                            "o (p c) -> o p c", p=P, c=4),
                        in_=r_coll[:, ih * 4:(ih + 1) * 4],
                    )
                rmb = work2.tile([D, N], f32, tag="rmb")
                nc.gpsimd.partition_broadcast(rmb, rm_row)
                hscr = work2.tile([D, N], f32r, tag="hscr")
                for ih in range(2):
                    t1 = work2.tile([D, 512], f32, tag="t1")
                    nc.vector.tensor_mul(t1, oa[ih][0:D, :], rmb[:, ts(ih, 512)])
                    nc.vector.scalar_tensor_tensor(
                        hscr[:, ts(ih, 512)], ommb[0:D, ts(ih, 512)],
                        mean_cols[:, h:h + 1], t1, OP.mult, OP.add)
                nc.sync.dma_start(
                    out=ot_dram[t * P + hoff: t * P + hoff + D, :], in_=hscr)

        # ---------- out projection ----------
        for doh in range(2):
            for kc in range(KT):
                whalf[kc] = work1.tile([P, 512], f32r, tag=f"whalf{kc}", name=f"whalf{kc}")
            for rt in range(4):
                wr = rowload.tile([P, DIM], f32, tag="rowload")
                nc.sync.dma_start(
                    out=wr,
                    in_=wout_d[doh * 512 + rt * P: doh * 512 + (rt + 1) * P, :])
                for kc in range(KT):
                    tp = ps_tp.tile([P, P], f32, tag="tp")
                    nc.tensor.transpose(tp, wr[:, ts(kc, P)], ident)
                    nc.vector.tensor_copy(whalf[kc][:, ts(rt, P)], tp)
            for nt in range(NT):
                acc = ps_mm.tile([P, 512], f32, tag="mm")
                for kc in range(KT):
                    otr = otrp.tile([P, P], f32r, tag="otr")
                    nc.sync.dma_start(out=otr, in_=ot_dram[ts(kc, P), ts(nt, P)])
                    nc.tensor.matmul(acc, otr, whalf[kc],
                                     start=(kc == 0), stop=(kc == KT - 1))
                ostage = work2.tile([P, 512], f32, tag="ostage")
                nc.vector.tensor_add(ostage, acc, b_bcast[:, ts(doh, 512)])
                nc.sync.dma_start(out=out_d[ts(nt, P), ts(doh, 512)], in_=ostage)

    nc.finalize()
    return nc


def kernel(x, mask, pos, W_qk, W_v, W_out, b_out):
    global _NC
    from concourse.bass_utils import run_bass_kernel_spmd

    if _NC is None:
        _NC = _build()

    x = np.ascontiguousarray(x, dtype=np.float32)
    pos = np.ascontiguousarray(pos, dtype=np.float32)
    maskf = np.concatenate(
        [np.ones((B, 1), np.float32), np.asarray(mask).astype(np.float32)], axis=1)
    W_qk = np.ascontiguousarray(W_qk, dtype=np.float32)
    W_v = np.ascontiguousarray(W_v, dtype=np.float32)
    W_out = np.ascontiguousarray(W_out, dtype=np.float32)
    b_out = np.ascontiguousarray(b_out, dtype=np.float32)

    in_maps = [
        {"x": x[b], "pos": pos[b], "maskf": maskf[b], "W_qk": W_qk,
         "W_v": W_v, "W_out": W_out, "b_out": b_out}
        for b in range(B)
    ]
    res = run_bass_kernel_spmd(_NC, in_maps, core_ids=list(range(B)))
    return np.stack([res.results[b]["out"] for b in range(B)]).astype(np.float32)
